# revision 4
# baseline (speedup 1.0000x reference)
"""Trainium2 Bass kernel for BatchedLUTNodes (v4).

Math: out[b,n] = sum_e tables[n,e] * prod_i (x_i*bit_i(e) + (1-x_i)*(1-bit_i(e)))
is 6-dim multilinear interpolation. In the monomial basis:
    out[b,n] = sum_{p,k} u_p[b,n] * C[n][p,k] * v_k[b,n]
with u = monomials of (x0,x1,x2) and v = monomials of (x3,x4,x5), each 8-wide
in slot order [1, a, b, c, ab, ac, bc, abc]; C[n] (8x8) is the Moebius
(finite-difference) transform of tables[n], computed on the host.

Pipeline per node-tile (128 nodes):
  PE 8x matmul (vt stationary, dense block-diag C moving) -> PSUM (G,p,g)
  ACT evacuates PSUM->SBUF fp16 with a strided write that reshuffles to
      p-major (p, node), so everything downstream is contiguous
  DVE z = y * u (2x fp16), GPSIMD does the first add-tree level,
  DVE the last two levels -> out columns; DMA out per tile-pair.

v4: ux is DMAd right after tile 0 so the u-monomial build (DVE) completes
before the pipeline fills; the first add-tree level runs on GPSIMD to keep
DVE under the DMA cadence; tile 7 is split into two half-tiles (all-DVE
tree) to shorten the post-stream drain; vt+cd are interleaved per tile in
one HBM tensor so each tile is a single 512KB DMA with one semaphore.

Sharding: nodes split 8 ways (1024/core), C sharded alongside.
"""

import numpy as np

try:
    from concourse import bass, tile
    from concourse import bass_utils
except ImportError:
    import sys
    sys.path.insert(0, "/opt/trn_rl_repo")
    from concourse import bass, tile
    from concourse import bass_utils

from concourse.tile import add_dep_helper

mybir = bass.mybir
F32 = mybir.dt.float32
F16 = mybir.dt.float16

B = 128            # batch (partition dim)
N = 8192           # total nodes
NCORES = 8
NPC = N // NCORES  # nodes per core = 1024
NT = 8             # node-tiles per core (128 nodes each)
TN = 128           # nodes per tile
NG = 8             # matmul groups per tile
GN = 16            # nodes per group
UC = 8 * NPC       # u row length


def build_nc() -> bass.Bass:
    nc = bass.Bass("TRN2", target_bir_lowering=False, debug=False)
    # ux: raw u-vars, j-major: col j*1024 + nl holds x_j[b, node nl]
    ux = nc.dram_tensor("ux", [B, 3 * NPC], F16, kind="ExternalInput")
    # vc: per tile t, cols [t*2048, t*2048+1024) = vt (host-transposed v
    # monomials, vc[8g+k, t*2048 + G*128 + b]), cols [t*2048+1024, ...) = cd
    # (dense slot-major block-diag C image, vc[8g+k, t*2048+1024 + G*128 +
    # p*16 + g2] = C[node(t,G,g)][p,k] iff g2==g).
    vc = nc.dram_tensor("vc", [128, NT * 2048], F16, kind="ExternalInput")
    out = nc.dram_tensor("out", [B, NPC], F16, kind="ExternalOutput")

    chain_prev = {}

    def chain(key, binst):
        # same-engine program-order edge: no semaphore cost, but keeps
        # the scheduler from reordering so sem-wait elision works
        prev = chain_prev.get(key)
        if prev is not None:
            add_dep_helper(binst.ins, prev, sync=False, reason=f"{key} order chain")
        chain_prev[key] = binst.ins
        return binst

    TT = mybir.AluOpType.mult
    TA = mybir.AluOpType.add

    with tile.TileContext(nc):
        # ---- persistent SBUF / PSUM regions ------------------------
        # U: u monomials, p-major: col p*1024 + nl = u_p[b, node nl]
        U = nc.alloc_sbuf_tensor("u_all", [B, UC], F16)
        vcs = nc.alloc_sbuf_tensor("vc_all", [128, NT * 2048], F16)
        ysb = nc.alloc_sbuf_tensor("ysb", [B, 3 * 1024], F16)   # 3 bufs
        zb = nc.alloc_sbuf_tensor("zb", [B, 2 * 1024], F16)     # 2 bufs
        osb = nc.alloc_sbuf_tensor("osb", [B, NPC], F16)
        yp = nc.alloc_psum_tensor("yp", [B, 3 * 1024], F32)     # 6 banks

        # ---- input DMAs (SP HWDGE ring, FIFO) ----------------------
        dma = nc.sync.dma_start

        def vc_dma(c0, c1):
            chain('SPD', dma(vcs[:, c0:c1], vc[:, c0:c1]))

        vc_dma(0, 2048)                        # tile 0
        chain('SPD', dma(bass.AP(U, NPC, [[UC, B], [1, 3 * NPC]]),
                         ux[:, :]))            # all three raw u-vars
        for t in range(1, NT - 1):
            vc_dma(t * 2048, (t + 1) * 2048)
        vc_dma(7 * 2048, 7 * 2048 + 1024)      # tile 7: vt
        vc_dma(7 * 2048 + 1024, 8 * 2048)      # tile 7: cd

        # ---- u monomials on DVE (p-major layout) -------------------
        uap = lambda p, d=1: bass.AP(U, p * NPC, [[UC, B], [NPC, d], [1, NPC]])
        chain('DVE', nc.vector.memset(
            bass.AP(U, 0, [[UC, B], [1, NPC]]), 1.0))
        chain('DVE', nc.vector.tensor_tensor(uap(4), uap(1), uap(2), TT))
        chain('DVE', nc.vector.tensor_tensor(
            uap(5, 2), uap(1, 2),
            bass.AP(U, 3 * NPC, [[UC, B], [0, 2], [1, NPC]]), TT))
        chain('DVE', nc.vector.tensor_tensor(uap(7), uap(4), uap(3), TT))

        for t in range(NT):
            pb = (t % 3) * 1024          # psum buf col offset
            yb = (t % 3) * 1024          # ysb buf col offset
            zo = (t % 2) * 1024          # zb buf col offset
            for G in range(NG):
                lhsT = vcs[:, t * 2048 + G * 128:t * 2048 + (G + 1) * 128]
                rhs = vcs[:, t * 2048 + 1024 + G * 128:
                          t * 2048 + 1024 + (G + 1) * 128]
                chain('PE', nc.tensor.matmul(
                    yp[:, pb + G * 128:pb + (G + 1) * 128],
                    lhsT=lhsT, rhs=rhs, start=True, stop=True))

            if t < NT - 1:
                # evacuate PSUM -> SBUF fp16 on ACT, reshuffling (G,p,g)
                # -> (p, G, g) = p-major so the DVE ops are contiguous.
                chain('ACT', nc.scalar.copy(
                    bass.AP(ysb, yb,
                            [[3 * 1024, B], [16, NG], [128, 8], [1, GN]]),
                    bass.AP(yp, pb, [[3 * 1024, B], [1, 1024]])))
                # z = y * u  (contiguous fp16 -> DVE 2x)
                chain('DVE', nc.vector.tensor_tensor(
                    zb[:, zo:zo + 1024], ysb[:, yb:yb + 1024],
                    bass.AP(U, t * TN, [[UC, B], [NPC, 8], [1, TN]]), TT))
                # add tree over p: L1 on GPSIMD, L2/L3 on DVE
                chain('POOL', nc.gpsimd.tensor_tensor(
                    zb[:, zo:zo + 512], zb[:, zo:zo + 512],
                    zb[:, zo + 512:zo + 1024], TA))
                chain('DVE', nc.vector.tensor_tensor(
                    zb[:, zo:zo + 256], zb[:, zo:zo + 256],
                    zb[:, zo + 256:zo + 512], TA))
                chain('DVE', nc.vector.tensor_tensor(
                    osb[:, t * TN:(t + 1) * TN], zb[:, zo:zo + 128],
                    zb[:, zo + 128:zo + 256], TA))
                if t % 2 == 1:
                    chain('SPD', dma(out[:, (t - 1) * TN:(t + 1) * TN],
                                     osb[:, (t - 1) * TN:(t + 1) * TN]))
                elif t == NT - 2:
                    chain('SPD', dma(out[:, t * TN:(t + 1) * TN],
                                     osb[:, t * TN:(t + 1) * TN]))
            else:
                # tile 7: two half-tiles, all-DVE tree, to shorten the
                # post-stream drain. Half h covers groups 4h..4h+4 =
                # nodes 64h..64h+64; evac is p-major over 64 nodes.
                for h in range(2):
                    chain('ACT', nc.scalar.copy(
                        bass.AP(ysb, yb + h * 512,
                                [[3 * 1024, B], [16, 4], [64, 8], [1, GN]]),
                        bass.AP(yp, pb + h * 512,
                                [[3 * 1024, B], [1, 512]])))
                    zh = zo + h * 512
                    chain('DVE', nc.vector.tensor_tensor(
                        zb[:, zh:zh + 512], ysb[:, yb + h * 512:yb + h * 512 + 512],
                        bass.AP(U, t * TN + h * 64,
                                [[UC, B], [NPC, 8], [1, 64]]), TT))
                    chain('DVE', nc.vector.tensor_tensor(
                        zb[:, zh:zh + 256], zb[:, zh:zh + 256],
                        zb[:, zh + 256:zh + 512], TA))
                    chain('DVE', nc.vector.tensor_tensor(
                        zb[:, zh:zh + 128], zb[:, zh:zh + 128],
                        zb[:, zh + 128:zh + 256], TA))
                    chain('DVE', nc.vector.tensor_tensor(
                        osb[:, t * TN + h * 64:t * TN + h * 64 + 64],
                        zb[:, zh:zh + 64], zb[:, zh + 64:zh + 128], TA))
                    chain('SPD', dma(
                        out[:, t * TN + h * 64:t * TN + h * 64 + 64],
                        osb[:, t * TN + h * 64:t * TN + h * 64 + 64]))

    _split_multiwait(nc)
    return nc


def _split_multiwait(nc):
    """Walrus allows ~one sync-wait per TPB instruction; hoist extra waits
    onto same-engine no-op carriers inserted just before."""
    for fn in nc.m.functions:
        for blk in fn.blocks:
            out = []
            changed = False
            for ins in blk.instructions:
                si = getattr(ins, "sync_info", None)
                waits = list(si.on_wait) if si is not None else []
                if len(waits) > 1:
                    changed = True
                    for i, w in enumerate(waits[:-1]):
                        out.append(mybir.InstNoOp(
                            name=f"{ins.name}-w{i}",
                            engine=ins.engine,
                            bass_nofuse=True,
                            ins=[], outs=[],
                            sync_info=mybir.SyncInfo(
                                on_wait=[w], on_update=[]),
                        ))
                    ins.sync_info = mybir.SyncInfo(
                        on_wait=[waits[-1]], on_update=list(si.on_update))
                out.append(ins)
            if changed:
                blk.instructions = out


# ---------------------------------------------------------------- host side

# slot order [1, a, b, c, ab, ac, bc, abc] -> monomial bitmask (bit0=a,...)
SLOT2MON = np.array([0, 1, 2, 4, 3, 5, 6, 7])


def _monomial_C(tables: np.ndarray) -> np.ndarray:
    """tables (N, 64) -> C (N, 8, 8) fp32 in slot order: C[n, p, k]."""
    c = np.asarray(tables, np.float64).reshape(-1, 2, 2, 2, 2, 2, 2)
    for ax in range(1, 7):
        lo = np.take(c, 0, axis=ax)
        hi = np.take(c, 1, axis=ax)
        c = np.stack([lo, hi - lo], axis=ax)
    # axes (n, m5, m4, m3, m2, m1, m0): flat index m5*32+...+m0
    cm = c.reshape(-1, 64)
    flat = np.zeros((8, 8), np.int64)
    for jm in range(8):
        for km in range(8):
            m0, m1, m2 = jm & 1, (jm >> 1) & 1, (jm >> 2) & 1
            m3, m4, m5 = km & 1, (km >> 1) & 1, (km >> 2) & 1
            flat[jm, km] = m5 * 32 + m4 * 16 + m3 * 8 + m2 * 4 + m1 * 2 + m0
    idx = flat[SLOT2MON][:, SLOT2MON]   # idx[p, k], slot-ordered
    return cm[:, idx].astype(np.float32)  # (N, 8, 8)


def _v_monomials(xv: np.ndarray) -> np.ndarray:
    """xv (..., 3) -> (..., 8) slot-order monomials [1,a,b,c,ab,ac,bc,abc]."""
    a, b, c = xv[..., 0], xv[..., 1], xv[..., 2]
    one = np.ones_like(a)
    return np.stack([one, a, b, c, a * b, a * c, b * c, a * b * c], axis=-1)


def make_in_maps(x: np.ndarray, tables: np.ndarray):
    x = np.clip(np.asarray(x, np.float32), 0.0, 1.0)
    C = _monomial_C(np.asarray(tables, np.float32))  # (N, 8, 8)
    in_maps = []
    for core in range(NCORES):
        sl = slice(core * NPC, (core + 1) * NPC)
        xs = x[:, sl, :]                            # (B, 1024, 6)

        # ux: [b, (j, nl)] j-major raw u-vars x0..x2
        uxc = np.ascontiguousarray(
            xs[:, :, 0:3].transpose(0, 2, 1).reshape(B, 3 * NPC)
        ).astype(np.float16)

        # vt: [8g+k, (t, G, b)] = v_k[b, node t*128+G*16+g]
        vmon = _v_monomials(xs[:, :, 3:6]).astype(np.float16)  # (B,1024,8)
        vm = vmon.reshape(B, NT, NG, GN, 8)          # (b, t, G, g, k)
        vtc = np.ascontiguousarray(
            vm.transpose(3, 4, 1, 2, 0)              # (g, k, t, G, b)
            .reshape(128, NT, 1024))

        # cd: dense slot-major block-diag image [8g+k, (t, G, p, g2)]
        Cc = C[sl].reshape(NT, NG, GN, 8, 8)         # (t, G, g, p, k)
        cdc = np.zeros((128, NT, NG, 8, GN), np.float16)
        for g in range(GN):
            cdc[8 * g:8 * (g + 1), :, :, :, g] = \
                Cc[:, :, g, :, :].transpose(3, 0, 1, 2).astype(np.float16)
        cdc = cdc.reshape(128, NT, 1024)

        # vc: per tile [vt | cd]
        vcc = np.ascontiguousarray(
            np.concatenate([vtc, cdc], axis=2).reshape(128, NT * 2048))

        in_maps.append({"ux": uxc, "vc": vcc})
    return in_maps


_NC_CACHE = None


def _get_nc():
    global _NC_CACHE
    if _NC_CACHE is None:
        _NC_CACHE = build_nc()
    return _NC_CACHE


def kernel(x: np.ndarray, tables: np.ndarray, _trace: bool = False):
    nc = _get_nc()
    in_maps = make_in_maps(x, tables)
    res = bass_utils.run_bass_kernel_spmd(
        nc, in_maps, core_ids=list(range(NCORES)), trace=_trace,
    )
    out = np.concatenate(
        [r["out"].astype(np.float32) for r in res.results], axis=1)
    if _trace:
        return out, res
    return out


# revision 9
# speedup vs baseline: 1.0492x; 1.0492x over previous
"""Trainium2 Bass kernel for BatchedLUTNodes (v4).

Math: out[b,n] = sum_e tables[n,e] * prod_i (x_i*bit_i(e) + (1-x_i)*(1-bit_i(e)))
is 6-dim multilinear interpolation. In the monomial basis:
    out[b,n] = sum_{p,k} u_p[b,n] * C[n][p,k] * v_k[b,n]
with u = monomials of (x0,x1,x2) and v = monomials of (x3,x4,x5), each 8-wide
in slot order [1, a, b, c, ab, ac, bc, abc]; C[n] (8x8) is the Moebius
(finite-difference) transform of tables[n], computed on the host.

Pipeline per node-tile (128 nodes):
  PE 8x matmul (vt stationary, dense block-diag C moving) -> PSUM (G,p,g)
  ACT evacuates PSUM->SBUF fp16 with a strided write that reshuffles to
      p-major (p, node), so everything downstream is contiguous
  DVE z = y * u (2x fp16), GPSIMD does the first add-tree level,
  DVE the last two levels -> out columns; DMA out per tile-pair.

v4: ux is DMAd right after tile 0 so the u-monomial build (DVE) completes
before the pipeline fills; the first add-tree level runs on GPSIMD to keep
DVE under the DMA cadence; tile 7 is split into two half-tiles (all-DVE
tree) to shorten the post-stream drain; vt+cd are interleaved per tile in
one HBM tensor so each tile is a single 512KB DMA with one semaphore.

Sharding: nodes split 8 ways (1024/core), C sharded alongside.
"""

import numpy as np

try:
    from concourse import bass, tile
    from concourse import bass_utils
except ImportError:
    import sys
    sys.path.insert(0, "/opt/trn_rl_repo")
    from concourse import bass, tile
    from concourse import bass_utils

from concourse.tile import add_dep_helper

mybir = bass.mybir
F32 = mybir.dt.float32
F16 = mybir.dt.float16

B = 128            # batch (partition dim)
N = 8192           # total nodes
NCORES = 8
NPC = N // NCORES  # nodes per core = 1024
NT = 8             # node-tiles per core (128 nodes each)
TN = 128           # nodes per tile
NG = 8             # matmul groups per tile
GN = 16            # nodes per group
UC = 8 * NPC       # u row length


def build_nc() -> bass.Bass:
    nc = bass.Bass("TRN2", target_bir_lowering=False, debug=False)
    # ux: raw u-vars, j-major: col j*1024 + nl holds x_j[b, node nl]
    ux = nc.dram_tensor("ux", [B, 3 * NPC], F16, kind="ExternalInput")
    # vc: per tile t, cols [t*2048, t*2048+1024) = vt (host-transposed v
    # monomials, vc[8g+k, t*2048 + G*128 + b]), cols [t*2048+1024, ...) = cd
    # (dense slot-major block-diag C image, vc[8g+k, t*2048+1024 + G*128 +
    # p*16 + g2] = C[node(t,G,g)][p,k] iff g2==g).
    vc = nc.dram_tensor("vc", [128, NT * 2048], F16, kind="ExternalInput")
    out = nc.dram_tensor("out", [B, NPC], F16, kind="ExternalOutput")

    chain_prev = {}

    def chain(key, binst):
        # same-engine program-order edge: no semaphore cost, but keeps
        # the scheduler from reordering so sem-wait elision works
        prev = chain_prev.get(key)
        if prev is not None:
            add_dep_helper(binst.ins, prev, sync=False, reason=f"{key} order chain")
        chain_prev[key] = binst.ins
        return binst

    TT = mybir.AluOpType.mult
    TA = mybir.AluOpType.add

    with tile.TileContext(nc):
        # ---- persistent SBUF / PSUM regions ------------------------
        # U: u monomials, p-major: col p*1024 + nl = u_p[b, node nl]
        U = nc.alloc_sbuf_tensor("u_all", [B, UC], F16)
        vcs = nc.alloc_sbuf_tensor("vc_all", [128, NT * 2048], F16)
        ysb = nc.alloc_sbuf_tensor("ysb", [B, 3 * 1024], F16)   # 3 bufs
        zb = nc.alloc_sbuf_tensor("zb", [B, 3 * 1024], F16)     # 3 bufs
        osb = nc.alloc_sbuf_tensor("osb", [B, NPC], F16)
        yp = nc.alloc_psum_tensor("yp", [B, 3 * 1024], F32)     # 6 banks

        # ---- input DMAs (SP HWDGE ring, FIFO) ----------------------
        dma = nc.sync.dma_start

        def vc_dma(c0, c1):
            chain('SPD', dma(vcs[:, c0:c1], vc[:, c0:c1]))

        vc_dma(0, 2048)                        # tile 0
        chain('SPD', dma(bass.AP(U, NPC, [[UC, B], [1, 2 * NPC]]),
                         ux[:, 0:2 * NPC]))    # x0, x1
        chain('SPD', dma(bass.AP(U, 3 * NPC, [[UC, B], [1, NPC]]),
                         ux[:, 2 * NPC:]))     # x2
        for t in range(1, NT - 1):
            vc_dma(t * 2048, (t + 1) * 2048)
        vc_dma(7 * 2048, 7 * 2048 + 1024)      # tile 7: vt
        vc_dma(7 * 2048 + 1024, 8 * 2048)      # tile 7: cd

        # ---- u monomials on DVE (p-major layout) -------------------
        uap = lambda p, d=1: bass.AP(U, p * NPC, [[UC, B], [NPC, d], [1, NPC]])
        chain('DVE', nc.vector.memset(
            bass.AP(U, 0, [[UC, B], [1, NPC]]), 1.0))
        chain('DVE', nc.vector.tensor_tensor(uap(4), uap(1), uap(2), TT))
        chain('DVE', nc.vector.tensor_tensor(
            uap(5, 2), uap(1, 2),
            bass.AP(U, 3 * NPC, [[UC, B], [0, 2], [1, NPC]]), TT))
        chain('DVE', nc.vector.tensor_tensor(uap(7), uap(4), uap(3), TT))

        # Software-pipelined emission: DVE does mult(t+1) while GPSIMD does
        # the L1 add of tile t, then DVE finishes L2/L3 of tile t. The DVE
        # program order is mult(0), mult(1), L2(0), L3(0), mult(2), L2(1)...
        # so the Pool hop never sits inside DVE's serial chain.
        def emit_mms(t):
            pb = (t % 3) * 1024
            for G in range(NG):
                lhsT = vcs[:, t * 2048 + G * 128:t * 2048 + (G + 1) * 128]
                rhs = vcs[:, t * 2048 + 1024 + G * 128:
                          t * 2048 + 1024 + (G + 1) * 128]
                chain('PE', nc.tensor.matmul(
                    yp[:, pb + G * 128:pb + (G + 1) * 128],
                    lhsT=lhsT, rhs=rhs, start=True, stop=True))

        def emit_evac(t):
            # PSUM -> SBUF fp16 on ACT, reshuffling (G,p,g) -> (p, G, g)
            # = p-major so the DVE ops below are contiguous.
            pb = (t % 3) * 1024
            chain('ACT', nc.scalar.copy(
                bass.AP(ysb, pb,
                        [[3 * 1024, B], [16, NG], [128, 8], [1, GN]]),
                bass.AP(yp, pb, [[3 * 1024, B], [1, 1024]])))

        def emit_mult_l1(t):
            yb = (t % 3) * 1024
            zo = (t % 3) * 1024
            chain('DVE', nc.vector.tensor_tensor(
                zb[:, zo:zo + 1024], ysb[:, yb:yb + 1024],
                bass.AP(U, t * TN, [[UC, B], [NPC, 8], [1, TN]]), TT))
            chain('POOL', nc.gpsimd.tensor_tensor(
                zb[:, zo:zo + 512], zb[:, zo:zo + 512],
                zb[:, zo + 512:zo + 1024], TA))

        def emit_l2_l3(t):
            zo = (t % 3) * 1024
            chain('DVE', nc.vector.tensor_tensor(
                zb[:, zo:zo + 256], zb[:, zo:zo + 256],
                zb[:, zo + 256:zo + 512], TA))
            chain('DVE', nc.vector.tensor_tensor(
                osb[:, t * TN:(t + 1) * TN], zb[:, zo:zo + 128],
                zb[:, zo + 128:zo + 256], TA))
            if t % 2 == 1:
                chain('SPD', dma(out[:, (t - 1) * TN:(t + 1) * TN],
                                 osb[:, (t - 1) * TN:(t + 1) * TN]))
            elif t == NT - 2:
                chain('SPD', dma(out[:, t * TN:(t + 1) * TN],
                                 osb[:, t * TN:(t + 1) * TN]))

        def emit_half7(h):
            # tile 7 half h: groups 4h..4h+4 = nodes 64h..64h+64, all-DVE
            # tree to shorten the post-stream drain.
            t = NT - 1
            pb = (t % 3) * 1024
            zh = (t % 3) * 1024 + h * 512
            chain('ACT', nc.scalar.copy(
                bass.AP(ysb, pb + h * 512,
                        [[3 * 1024, B], [16, 4], [64, 8], [1, GN]]),
                bass.AP(yp, pb + h * 512, [[3 * 1024, B], [1, 512]])))
            chain('DVE', nc.vector.tensor_tensor(
                zb[:, zh:zh + 512], ysb[:, pb + h * 512:pb + h * 512 + 512],
                bass.AP(U, t * TN + h * 64,
                        [[UC, B], [NPC, 8], [1, 64]]), TT))
            chain('DVE', nc.vector.tensor_tensor(
                zb[:, zh:zh + 256], zb[:, zh:zh + 256],
                zb[:, zh + 256:zh + 512], TA))
            chain('DVE', nc.vector.tensor_tensor(
                zb[:, zh:zh + 128], zb[:, zh:zh + 128],
                zb[:, zh + 128:zh + 256], TA))
            chain('DVE', nc.vector.tensor_tensor(
                osb[:, t * TN + h * 64:t * TN + h * 64 + 64],
                zb[:, zh:zh + 64], zb[:, zh + 64:zh + 128], TA))
            chain('SPD', dma(
                out[:, t * TN + h * 64:t * TN + h * 64 + 64],
                osb[:, t * TN + h * 64:t * TN + h * 64 + 64]))

        for t in range(NT):
            emit_mms(t)
            if t < NT - 1:
                emit_evac(t)
                emit_mult_l1(t)
            if t >= 2:
                emit_l2_l3(t - 2)
        emit_l2_l3(NT - 2)
        emit_half7(0)
        emit_half7(1)

    _split_multiwait(nc)
    return nc


def _split_multiwait(nc):
    """Walrus allows ~one sync-wait per TPB instruction; hoist extra waits
    onto same-engine no-op carriers inserted just before."""
    for fn in nc.m.functions:
        for blk in fn.blocks:
            out = []
            changed = False
            for ins in blk.instructions:
                si = getattr(ins, "sync_info", None)
                waits = list(si.on_wait) if si is not None else []
                if len(waits) > 1:
                    changed = True
                    for i, w in enumerate(waits[:-1]):
                        out.append(mybir.InstNoOp(
                            name=f"{ins.name}-w{i}",
                            engine=ins.engine,
                            bass_nofuse=True,
                            ins=[], outs=[],
                            sync_info=mybir.SyncInfo(
                                on_wait=[w], on_update=[]),
                        ))
                    ins.sync_info = mybir.SyncInfo(
                        on_wait=[waits[-1]], on_update=list(si.on_update))
                out.append(ins)
            if changed:
                blk.instructions = out


# ---------------------------------------------------------------- host side

# slot order [1, a, b, c, ab, ac, bc, abc] -> monomial bitmask (bit0=a,...)
SLOT2MON = np.array([0, 1, 2, 4, 3, 5, 6, 7])


def _monomial_C(tables: np.ndarray) -> np.ndarray:
    """tables (N, 64) -> C (N, 8, 8) fp32 in slot order: C[n, p, k]."""
    c = np.asarray(tables, np.float64).reshape(-1, 2, 2, 2, 2, 2, 2)
    for ax in range(1, 7):
        lo = np.take(c, 0, axis=ax)
        hi = np.take(c, 1, axis=ax)
        c = np.stack([lo, hi - lo], axis=ax)
    # axes (n, m5, m4, m3, m2, m1, m0): flat index m5*32+...+m0
    cm = c.reshape(-1, 64)
    flat = np.zeros((8, 8), np.int64)
    for jm in range(8):
        for km in range(8):
            m0, m1, m2 = jm & 1, (jm >> 1) & 1, (jm >> 2) & 1
            m3, m4, m5 = km & 1, (km >> 1) & 1, (km >> 2) & 1
            flat[jm, km] = m5 * 32 + m4 * 16 + m3 * 8 + m2 * 4 + m1 * 2 + m0
    idx = flat[SLOT2MON][:, SLOT2MON]   # idx[p, k], slot-ordered
    return cm[:, idx].astype(np.float32)  # (N, 8, 8)


def _v_monomials(xv: np.ndarray) -> np.ndarray:
    """xv (..., 3) -> (..., 8) slot-order monomials [1,a,b,c,ab,ac,bc,abc]."""
    a, b, c = xv[..., 0], xv[..., 1], xv[..., 2]
    one = np.ones_like(a)
    return np.stack([one, a, b, c, a * b, a * c, b * c, a * b * c], axis=-1)


def make_in_maps(x: np.ndarray, tables: np.ndarray):
    x = np.clip(np.asarray(x, np.float32), 0.0, 1.0)
    C = _monomial_C(np.asarray(tables, np.float32))  # (N, 8, 8)
    in_maps = []
    for core in range(NCORES):
        sl = slice(core * NPC, (core + 1) * NPC)
        xs = x[:, sl, :]                            # (B, 1024, 6)

        # ux: [b, (j, nl)] j-major raw u-vars x0..x2
        uxc = np.ascontiguousarray(
            xs[:, :, 0:3].transpose(0, 2, 1).reshape(B, 3 * NPC)
        ).astype(np.float16)

        # vt: [8g+k, (t, G, b)] = v_k[b, node t*128+G*16+g]
        vmon = _v_monomials(xs[:, :, 3:6]).astype(np.float16)  # (B,1024,8)
        vm = vmon.reshape(B, NT, NG, GN, 8)          # (b, t, G, g, k)
        vtc = np.ascontiguousarray(
            vm.transpose(3, 4, 1, 2, 0)              # (g, k, t, G, b)
            .reshape(128, NT, 1024))

        # cd: dense slot-major block-diag image [8g+k, (t, G, p, g2)]
        Cc = C[sl].reshape(NT, NG, GN, 8, 8)         # (t, G, g, p, k)
        cdc = np.zeros((128, NT, NG, 8, GN), np.float16)
        for g in range(GN):
            cdc[8 * g:8 * (g + 1), :, :, :, g] = \
                Cc[:, :, g, :, :].transpose(3, 0, 1, 2).astype(np.float16)
        cdc = cdc.reshape(128, NT, 1024)

        # vc: per tile [vt | cd]
        vcc = np.ascontiguousarray(
            np.concatenate([vtc, cdc], axis=2).reshape(128, NT * 2048))

        in_maps.append({"ux": uxc, "vc": vcc})
    return in_maps


_NC_CACHE = None


def _get_nc():
    global _NC_CACHE
    if _NC_CACHE is None:
        _NC_CACHE = build_nc()
    return _NC_CACHE


def kernel(x: np.ndarray, tables: np.ndarray, _trace: bool = False):
    nc = _get_nc()
    in_maps = make_in_maps(x, tables)
    res = bass_utils.run_bass_kernel_spmd(
        nc, in_maps, core_ids=list(range(NCORES)), trace=_trace,
    )
    out = np.concatenate(
        [r["out"].astype(np.float32) for r in res.results], axis=1)
    if _trace:
        return out, res
    return out


# revision 11
# speedup vs baseline: 1.0632x; 1.0133x over previous
"""Trainium2 Bass kernel for BatchedLUTNodes (v4).

Math: out[b,n] = sum_e tables[n,e] * prod_i (x_i*bit_i(e) + (1-x_i)*(1-bit_i(e)))
is 6-dim multilinear interpolation. In the monomial basis:
    out[b,n] = sum_{p,k} u_p[b,n] * C[n][p,k] * v_k[b,n]
with u = monomials of (x0,x1,x2) and v = monomials of (x3,x4,x5), each 8-wide
in slot order [1, a, b, c, ab, ac, bc, abc]; C[n] (8x8) is the Moebius
(finite-difference) transform of tables[n], computed on the host.

Pipeline per node-tile (128 nodes):
  PE 8x matmul (vt stationary, dense block-diag C moving) -> PSUM (G,p,g)
  ACT evacuates PSUM->SBUF fp16 with a strided write that reshuffles to
      p-major (p, node), so everything downstream is contiguous
  DVE z = y * u (2x fp16), GPSIMD does the first add-tree level,
  DVE the last two levels -> out columns; DMA out per tile-pair.

v4: ux is DMAd right after tile 0 so the u-monomial build (DVE) completes
before the pipeline fills; the first add-tree level runs on GPSIMD to keep
DVE under the DMA cadence; tile 7 is split into two half-tiles (all-DVE
tree) to shorten the post-stream drain; vt+cd are interleaved per tile in
one HBM tensor so each tile is a single 512KB DMA with one semaphore.

Sharding: nodes split 8 ways (1024/core), C sharded alongside.
"""

import numpy as np

try:
    from concourse import bass, tile
    from concourse import bass_utils
except ImportError:
    import sys
    sys.path.insert(0, "/opt/trn_rl_repo")
    from concourse import bass, tile
    from concourse import bass_utils

from concourse.tile import add_dep_helper

mybir = bass.mybir
F32 = mybir.dt.float32
F16 = mybir.dt.float16

B = 128            # batch (partition dim)
N = 8192           # total nodes
NCORES = 8
NPC = N // NCORES  # nodes per core = 1024
NT = 8             # node-tiles per core (128 nodes each)
TN = 128           # nodes per tile
NG = 8             # matmul groups per tile
GN = 16            # nodes per group
UC = 8 * NPC       # u row length


def build_nc() -> bass.Bass:
    nc = bass.Bass("TRN2", target_bir_lowering=False, debug=False)
    # ux: raw u-vars, j-major: col j*1024 + nl holds x_j[b, node nl]
    ux = nc.dram_tensor("ux", [B, 3 * NPC], F16, kind="ExternalInput")
    # vc: per tile t, cols [t*2048, t*2048+1024) = vt (host-transposed v
    # monomials, vc[8g+k, t*2048 + G*128 + b]), cols [t*2048+1024, ...) = cd
    # (dense slot-major block-diag C image, vc[8g+k, t*2048+1024 + G*128 +
    # p*16 + g2] = C[node(t,G,g)][p,k] iff g2==g).
    vc = nc.dram_tensor("vc", [128, NT * 2048], F16, kind="ExternalInput")
    out = nc.dram_tensor("out", [B, NPC], F16, kind="ExternalOutput")

    chain_prev = {}

    def chain(key, binst):
        # same-engine program-order edge: no semaphore cost, but keeps
        # the scheduler from reordering so sem-wait elision works
        prev = chain_prev.get(key)
        if prev is not None:
            add_dep_helper(binst.ins, prev, sync=False, reason=f"{key} order chain")
        chain_prev[key] = binst.ins
        return binst

    TT = mybir.AluOpType.mult
    TA = mybir.AluOpType.add

    with tile.TileContext(nc):
        # ---- persistent SBUF / PSUM regions ------------------------
        # U: u monomials, p-major: col p*1024 + nl = u_p[b, node nl]
        U = nc.alloc_sbuf_tensor("u_all", [B, UC], F16)
        vcs = nc.alloc_sbuf_tensor("vc_all", [128, NT * 2048], F16)
        ysb = nc.alloc_sbuf_tensor("ysb", [B, 3 * 1024], F16)   # 3 bufs
        zb = nc.alloc_sbuf_tensor("zb", [B, 3 * 1024], F16)     # 3 bufs
        osb = nc.alloc_sbuf_tensor("osb", [B, NPC], F16)
        yp = nc.alloc_psum_tensor("yp", [B, 3 * 1024], F32)     # 6 banks
        wsrc = nc.alloc_sbuf_tensor("wsrc", [B, 128], F16)
        wp = nc.alloc_psum_tensor("wp", [B, 128], F32)          # 7th bank

        # ---- input DMAs (SP HWDGE ring, FIFO) ----------------------
        dma = nc.sync.dma_start

        def vc_dma(c0, c1):
            chain('SPD', dma(vcs[:, c0:c1], vc[:, c0:c1]))

        vc_dma(0, 2048)                        # tile 0
        chain('SPD', dma(bass.AP(U, NPC, [[UC, B], [1, 2 * NPC]]),
                         ux[:, 0:2 * NPC]))    # x0, x1
        chain('SPD', dma(bass.AP(U, 3 * NPC, [[UC, B], [1, NPC]]),
                         ux[:, 2 * NPC:]))     # x2
        for t in range(1, NT - 1):
            vc_dma(t * 2048, (t + 1) * 2048)
        vc_dma(7 * 2048, 7 * 2048 + 1024)      # tile 7: vt
        vc_dma(7 * 2048 + 1024, 8 * 2048)      # tile 7: cd

        # ---- PE warm-up: ~16 dummy matmuls while the first DMA lands,
        # so the HAM clock-gate un-throttles (1.2 -> 2.4 GHz) before the
        # real per-tile matmuls start.
        chain('DVE', nc.vector.memset(wsrc[:, :], 1.0))
        for _ in range(16):
            chain('PE', nc.tensor.matmul(
                wp[:, :], lhsT=wsrc[:, :], rhs=wsrc[:, :],
                start=True, stop=True))

        # ---- u monomials on DVE (p-major layout) -------------------
        uap = lambda p, d=1: bass.AP(U, p * NPC, [[UC, B], [NPC, d], [1, NPC]])
        chain('DVE', nc.vector.memset(
            bass.AP(U, 0, [[UC, B], [1, NPC]]), 1.0))
        chain('DVE', nc.vector.tensor_tensor(uap(4), uap(1), uap(2), TT))
        chain('DVE', nc.vector.tensor_tensor(
            uap(5, 2), uap(1, 2),
            bass.AP(U, 3 * NPC, [[UC, B], [0, 2], [1, NPC]]), TT))
        chain('DVE', nc.vector.tensor_tensor(uap(7), uap(4), uap(3), TT))

        # Software-pipelined emission: DVE does mult(t+1) while GPSIMD does
        # the L1 add of tile t, then DVE finishes L2/L3 of tile t. The DVE
        # program order is mult(0), mult(1), L2(0), L3(0), mult(2), L2(1)...
        # so the Pool hop never sits inside DVE's serial chain.
        def emit_mms(t):
            pb = (t % 3) * 1024
            for G in range(NG):
                lhsT = vcs[:, t * 2048 + G * 128:t * 2048 + (G + 1) * 128]
                rhs = vcs[:, t * 2048 + 1024 + G * 128:
                          t * 2048 + 1024 + (G + 1) * 128]
                chain('PE', nc.tensor.matmul(
                    yp[:, pb + G * 128:pb + (G + 1) * 128],
                    lhsT=lhsT, rhs=rhs, start=True, stop=True))

        def emit_evac(t):
            # PSUM -> SBUF fp16 on ACT, reshuffling (G,p,g) -> (p, G, g)
            # = p-major so the DVE ops below are contiguous.
            pb = (t % 3) * 1024
            chain('ACT', nc.scalar.copy(
                bass.AP(ysb, pb,
                        [[3 * 1024, B], [16, NG], [128, 8], [1, GN]]),
                bass.AP(yp, pb, [[3 * 1024, B], [1, 1024]])))

        def emit_mult_l1(t):
            yb = (t % 3) * 1024
            zo = (t % 3) * 1024
            chain('DVE', nc.vector.tensor_tensor(
                zb[:, zo:zo + 1024], ysb[:, yb:yb + 1024],
                bass.AP(U, t * TN, [[UC, B], [NPC, 8], [1, TN]]), TT))
            chain('POOL', nc.gpsimd.tensor_tensor(
                zb[:, zo:zo + 512], zb[:, zo:zo + 512],
                zb[:, zo + 512:zo + 1024], TA))

        def emit_l2_l3(t):
            zo = (t % 3) * 1024
            chain('DVE', nc.vector.tensor_tensor(
                zb[:, zo:zo + 256], zb[:, zo:zo + 256],
                zb[:, zo + 256:zo + 512], TA))
            chain('DVE', nc.vector.tensor_tensor(
                osb[:, t * TN:(t + 1) * TN], zb[:, zo:zo + 128],
                zb[:, zo + 128:zo + 256], TA))
            if t % 2 == 1:
                chain('SPD', dma(out[:, (t - 1) * TN:(t + 1) * TN],
                                 osb[:, (t - 1) * TN:(t + 1) * TN]))
            elif t == NT - 2:
                chain('SPD', dma(out[:, t * TN:(t + 1) * TN],
                                 osb[:, t * TN:(t + 1) * TN]))

        def emit_half7(h):
            # tile 7 half h: groups 4h..4h+4 = nodes 64h..64h+64, all-DVE
            # tree to shorten the post-stream drain.
            t = NT - 1
            pb = (t % 3) * 1024
            zh = (t % 3) * 1024 + h * 512
            chain('ACT', nc.scalar.copy(
                bass.AP(ysb, pb + h * 512,
                        [[3 * 1024, B], [16, 4], [64, 8], [1, GN]]),
                bass.AP(yp, pb + h * 512, [[3 * 1024, B], [1, 512]])))
            chain('DVE', nc.vector.tensor_tensor(
                zb[:, zh:zh + 512], ysb[:, pb + h * 512:pb + h * 512 + 512],
                bass.AP(U, t * TN + h * 64,
                        [[UC, B], [NPC, 8], [1, 64]]), TT))
            chain('DVE', nc.vector.tensor_tensor(
                zb[:, zh:zh + 256], zb[:, zh:zh + 256],
                zb[:, zh + 256:zh + 512], TA))
            chain('DVE', nc.vector.tensor_tensor(
                zb[:, zh:zh + 128], zb[:, zh:zh + 128],
                zb[:, zh + 128:zh + 256], TA))
            chain('DVE', nc.vector.tensor_tensor(
                osb[:, t * TN + h * 64:t * TN + h * 64 + 64],
                zb[:, zh:zh + 64], zb[:, zh + 64:zh + 128], TA))
            chain('SPD', dma(
                out[:, t * TN + h * 64:t * TN + h * 64 + 64],
                osb[:, t * TN + h * 64:t * TN + h * 64 + 64]))

        for t in range(NT):
            emit_mms(t)
            if t < NT - 1:
                emit_evac(t)
                emit_mult_l1(t)
            if t >= 2:
                emit_l2_l3(t - 2)
        emit_l2_l3(NT - 2)
        emit_half7(0)
        emit_half7(1)

    _split_multiwait(nc)
    return nc


def _split_multiwait(nc):
    """Walrus allows ~one sync-wait per TPB instruction; hoist extra waits
    onto same-engine no-op carriers inserted just before."""
    for fn in nc.m.functions:
        for blk in fn.blocks:
            out = []
            changed = False
            for ins in blk.instructions:
                si = getattr(ins, "sync_info", None)
                waits = list(si.on_wait) if si is not None else []
                if len(waits) > 1:
                    changed = True
                    for i, w in enumerate(waits[:-1]):
                        out.append(mybir.InstNoOp(
                            name=f"{ins.name}-w{i}",
                            engine=ins.engine,
                            bass_nofuse=True,
                            ins=[], outs=[],
                            sync_info=mybir.SyncInfo(
                                on_wait=[w], on_update=[]),
                        ))
                    ins.sync_info = mybir.SyncInfo(
                        on_wait=[waits[-1]], on_update=list(si.on_update))
                out.append(ins)
            if changed:
                blk.instructions = out


# ---------------------------------------------------------------- host side

# slot order [1, a, b, c, ab, ac, bc, abc] -> monomial bitmask (bit0=a,...)
SLOT2MON = np.array([0, 1, 2, 4, 3, 5, 6, 7])


def _monomial_C(tables: np.ndarray) -> np.ndarray:
    """tables (N, 64) -> C (N, 8, 8) fp32 in slot order: C[n, p, k]."""
    c = np.asarray(tables, np.float64).reshape(-1, 2, 2, 2, 2, 2, 2)
    for ax in range(1, 7):
        lo = np.take(c, 0, axis=ax)
        hi = np.take(c, 1, axis=ax)
        c = np.stack([lo, hi - lo], axis=ax)
    # axes (n, m5, m4, m3, m2, m1, m0): flat index m5*32+...+m0
    cm = c.reshape(-1, 64)
    flat = np.zeros((8, 8), np.int64)
    for jm in range(8):
        for km in range(8):
            m0, m1, m2 = jm & 1, (jm >> 1) & 1, (jm >> 2) & 1
            m3, m4, m5 = km & 1, (km >> 1) & 1, (km >> 2) & 1
            flat[jm, km] = m5 * 32 + m4 * 16 + m3 * 8 + m2 * 4 + m1 * 2 + m0
    idx = flat[SLOT2MON][:, SLOT2MON]   # idx[p, k], slot-ordered
    return cm[:, idx].astype(np.float32)  # (N, 8, 8)


def _v_monomials(xv: np.ndarray) -> np.ndarray:
    """xv (..., 3) -> (..., 8) slot-order monomials [1,a,b,c,ab,ac,bc,abc]."""
    a, b, c = xv[..., 0], xv[..., 1], xv[..., 2]
    one = np.ones_like(a)
    return np.stack([one, a, b, c, a * b, a * c, b * c, a * b * c], axis=-1)


def make_in_maps(x: np.ndarray, tables: np.ndarray):
    x = np.clip(np.asarray(x, np.float32), 0.0, 1.0)
    C = _monomial_C(np.asarray(tables, np.float32))  # (N, 8, 8)
    in_maps = []
    for core in range(NCORES):
        sl = slice(core * NPC, (core + 1) * NPC)
        xs = x[:, sl, :]                            # (B, 1024, 6)

        # ux: [b, (j, nl)] j-major raw u-vars x0..x2
        uxc = np.ascontiguousarray(
            xs[:, :, 0:3].transpose(0, 2, 1).reshape(B, 3 * NPC)
        ).astype(np.float16)

        # vt: [8g+k, (t, G, b)] = v_k[b, node t*128+G*16+g]
        vmon = _v_monomials(xs[:, :, 3:6]).astype(np.float16)  # (B,1024,8)
        vm = vmon.reshape(B, NT, NG, GN, 8)          # (b, t, G, g, k)
        vtc = np.ascontiguousarray(
            vm.transpose(3, 4, 1, 2, 0)              # (g, k, t, G, b)
            .reshape(128, NT, 1024))

        # cd: dense slot-major block-diag image [8g+k, (t, G, p, g2)]
        Cc = C[sl].reshape(NT, NG, GN, 8, 8)         # (t, G, g, p, k)
        cdc = np.zeros((128, NT, NG, 8, GN), np.float16)
        for g in range(GN):
            cdc[8 * g:8 * (g + 1), :, :, :, g] = \
                Cc[:, :, g, :, :].transpose(3, 0, 1, 2).astype(np.float16)
        cdc = cdc.reshape(128, NT, 1024)

        # vc: per tile [vt | cd]
        vcc = np.ascontiguousarray(
            np.concatenate([vtc, cdc], axis=2).reshape(128, NT * 2048))

        in_maps.append({"ux": uxc, "vc": vcc})
    return in_maps


_NC_CACHE = None


def _get_nc():
    global _NC_CACHE
    if _NC_CACHE is None:
        _NC_CACHE = build_nc()
    return _NC_CACHE


def kernel(x: np.ndarray, tables: np.ndarray, _trace: bool = False):
    nc = _get_nc()
    in_maps = make_in_maps(x, tables)
    res = bass_utils.run_bass_kernel_spmd(
        nc, in_maps, core_ids=list(range(NCORES)), trace=_trace,
    )
    out = np.concatenate(
        [r["out"].astype(np.float32) for r in res.results], axis=1)
    if _trace:
        return out, res
    return out
